# revision 42
# baseline (speedup 1.0000x reference)
"""Trainium2 Bass kernel for the seq2seq GRU (encoder/decoder + vocab logits).

Strategy (8 NeuronCores):
  - The tiny GRU scans (B=32, H=128) are replicated on every core.
  - The dominant cost — the vocab projection [2016,128] @ [128,32000] and its
    ~262 MB f32 output write — is sharded over the vocab dim: each core owns
    V/8 = 4000 columns and writes its slab [32, 63, 4000] of the final output
    directly.
  - Decoder hidden states are stored in one tile per 4 timesteps so the vocab
    projection (float32r matmuls) and the output DMA stream overlap with the
    rest of the decoder scan.
  - r/z gate biases are folded into the input-side matmuls via an augmented
    ones-row; (1-z) comes from a second sigmoid with scale=-1.
  - Host only does embedding gathers, weight transposes, sharding and the
    constant first-column (BOS one-hot log) of the output.
"""

import numpy as np

EOS_IX = 2
BOS_IX = 1
V = 32000
E = 64
H = 128
B = 32
L = 64            # encoder length and output length
TD = L - 1        # 63 decoder steps
M = B * TD        # 2016 decoder tokens
NCORES = 8
VS = V // NCORES  # 4000
VW = 500          # vocab tile width (4000 = 8 * 500)
NVB = VS // VW    # 8
CH = 4            # decoder steps per chunk
NCHUNK = (TD + CH - 1) // CH          # 16 (last chunk has 3 steps)

_CACHE = {}


def _build():
    import concourse.bass as bass
    import concourse.mybir as mybir
    import concourse.tile as tile
    from concourse import bacc

    f32 = mybir.dt.float32
    f32r = mybir.dt.float32r
    AF = mybir.ActivationFunctionType
    ALU = mybir.AluOpType

    nc = bacc.Bacc(None, target_bir_lowering=False)

    xTa = nc.dram_tensor("xTa", [E + 1, L * B], f32, kind="ExternalInput")
    yTa = nc.dram_tensor("yTa", [E + 1, TD * B], f32, kind="ExternalInput")
    mask1 = nc.dram_tensor("mask1", [1, L * B], f32, kind="ExternalInput")
    eWrz = nc.dram_tensor("eWrz", [E + 1, 2 * H], f32, kind="ExternalInput")
    dWrz = nc.dram_tensor("dWrz", [E + 1, 2 * H], f32, kind="ExternalInput")
    eWn = nc.dram_tensor("eWn", [E, H], f32, kind="ExternalInput")
    dWn = nc.dram_tensor("dWn", [E, H], f32, kind="ExternalInput")
    eWhhT = nc.dram_tensor("eWhhT", [H, 3 * H], f32, kind="ExternalInput")
    dWhhT = nc.dram_tensor("dWhhT", [H, 3 * H], f32, kind="ExternalInput")
    dsWT = nc.dram_tensor("dsWT", [H, H], f32, kind="ExternalInput")
    biases = nc.dram_tensor("biases", [H, 5], f32, kind="ExternalInput")
    logWT = nc.dram_tensor("logWT", [H, VS], f32r, kind="ExternalInput")
    logb1 = nc.dram_tensor("logb1", [1, VS], f32, kind="ExternalInput")
    out = nc.dram_tensor("o", [M, VS], f32, kind="ExternalOutput")
    out_v = out.rearrange("(b t) v -> b t v", t=TD)   # [B, TD, VS]

    def bcast_ap(dram_t, n):
        # read the single row of dram_t into all 128 partitions (stride-0)
        ap = dram_t[:]
        return bass.AP(tensor=ap.tensor, offset=ap.offset, ap=[[0, 128], [1, n]])

    with tile.TileContext(nc) as tc:
        with (
            tc.tile_pool(name="state", bufs=1) as state,
            tc.tile_pool(name="gates", bufs=8) as gates,
            tc.tile_pool(name="stage", bufs=32) as stage,
            tc.tile_pool(name="psr", bufs=3, space="PSUM") as psr,
            tc.tile_pool(name="psn", bufs=3, space="PSUM") as psn,
            tc.tile_pool(name="pbig", bufs=2, space="PSUM") as pbig,
        ):
            # ---- load everything ----
            s_xTa = state.tile([E + 1, L * B], f32, tag="s_xTa")
            s_yTa = state.tile([E + 1, TD * B], f32, tag="s_yTa")
            s_mask = state.tile([128, L * B], f32, tag="s_mask")
            s_eWrz = state.tile([E + 1, 2 * H], f32, tag="s_eWrz")
            s_dWrz = state.tile([E + 1, 2 * H], f32, tag="s_dWrz")
            s_eWn = state.tile([E, H], f32, tag="s_eWn")
            s_dWn = state.tile([E, H], f32, tag="s_dWn")
            s_eWhhT = state.tile([H, 3 * H], f32, tag="s_eWhhT")
            s_dWhhT = state.tile([H, 3 * H], f32, tag="s_dWhhT")
            s_dsWT = state.tile([H, H], f32, tag="s_dsWT")
            s_bias = state.tile([H, 5], f32, tag="s_bias")
            s_logWT = state.tile([H, VS], f32r, tag="s_logWT")
            s_logb = state.tile([128, VS], f32, tag="s_logb")
            nc.sync.dma_start(s_xTa[:], xTa[:])
            nc.sync.dma_start(s_yTa[:], yTa[:])
            nc.sync.dma_start(s_mask[:], bcast_ap(mask1, L * B))
            nc.sync.dma_start(s_eWrz[:], eWrz[:])
            nc.sync.dma_start(s_dWrz[:], dWrz[:])
            nc.sync.dma_start(s_eWn[:], eWn[:])
            nc.sync.dma_start(s_dWn[:], dWn[:])
            nc.sync.dma_start(s_eWhhT[:], eWhhT[:])
            nc.sync.dma_start(s_dWhhT[:], dWhhT[:])
            nc.sync.dma_start(s_dsWT[:], dsWT[:])
            nc.sync.dma_start(s_bias[:], biases[:])
            nc.sync.dma_start(s_logWT[:], logWT[:])
            nc.sync.dma_start(s_logb[:], bcast_ap(logb1, VS))

            # bias cols: 0 enc bhh_n, 1 enc bih_n, 2 dec bhh_n, 3 dec bih_n, 4 ds_b
            def bcol(k):
                return s_bias[:, k : k + 1]

            # ---- precompute n-gate input-side activations xg_n = Wih_n @ x + bih_n
            s_xgn = state.tile([H, L * B], f32, tag="s_xgn")
            s_ygn = state.tile([H, TD * B], f32, tag="s_ygn")
            for nb in range(4):
                p = pbig.tile([H, 512], f32, tag="pbig")
                nc.tensor.matmul(
                    p[:], s_eWn[:], s_xTa[:E, nb * 512 : (nb + 1) * 512],
                    start=True, stop=True,
                )
                nc.scalar.activation(
                    s_xgn[:, nb * 512 : (nb + 1) * 512], p[:], AF.Identity,
                    bias=bcol(1),
                )
            for nb in range(4):
                p = pbig.tile([H, 504], f32, tag="pbig")
                nc.tensor.matmul(
                    p[:], s_dWn[:], s_yTa[:E, nb * 504 : (nb + 1) * 504],
                    start=True, stop=True,
                )
                nc.scalar.activation(
                    s_ygn[:, nb * 504 : (nb + 1) * 504], p[:], AF.Identity,
                    bias=bcol(3),
                )

            # ---- GRU scans ----
            s_encT = state.tile([H, L * B], f32, tag="s_encT")
            s_h0 = state.tile([H, B], f32, tag="s_h0")
            nc.vector.memset(s_h0[:], 0.0)

            def gru_step(h_parts, h_mat, wrz, whhT, xa, xgn, t, b_hn, h_out):
                """h_out <- GRU(h, x_t). The previous state enters the matmuls
                as its two partial products (z*h, (1-z)*n) so the final h'
                materialization stays off the recurrence-critical path.
                h_parts: rhs list for the Whh matmuls; h_mat: materialized h."""
                x_t = xa[:, t * B : (t + 1) * B]
                p_rz = psr.tile([H, 2 * B], f32, tag="p_rz")
                p_n = psn.tile([H, B], f32, tag="p_n")
                # input-side parts + folded biases (off critical path)
                nc.tensor.matmul(p_rz[:, 0:B], wrz[:, 0:H], x_t, start=True, stop=False)
                nc.tensor.matmul(p_rz[:, B : 2 * B], wrz[:, H : 2 * H], x_t, start=True, stop=False)
                # recurrent parts accumulate on top (one per h-part)
                for i, hp in enumerate(h_parts):
                    last = i == len(h_parts) - 1
                    nc.tensor.matmul(p_rz[:, 0:B], whhT[:, 0:H], hp, start=False, stop=last)
                    nc.tensor.matmul(p_rz[:, B : 2 * B], whhT[:, H : 2 * H], hp, start=False, stop=last)
                    nc.tensor.matmul(p_n[:], whhT[:, 2 * H : 3 * H], hp, start=(i == 0), stop=last)
                t_rz = gates.tile([H, 2 * B], f32, tag="t_rz")
                t_z2 = gates.tile([H, B], f32, tag="t_z2")
                t_zh = gates.tile([H, B], f32, tag="t_zh")
                t_nv = gates.tile([H, B], f32, tag="t_nv")
                t_ni = gates.tile([H, B], f32, tag="t_ni")
                t_n = gates.tile([H, B], f32, tag="t_n")
                t_z2n = gates.tile([H, B], f32, tag="t_z2n")
                # r|z = sigmoid(p_rz);  z2 = 1-z = sigmoid(-p_z)
                nc.scalar.activation(t_rz[:], p_rz[:], AF.Sigmoid)
                nc.scalar.activation(t_z2[:], p_rz[:, B : 2 * B], AF.Sigmoid, scale=-1.0)
                # chain ops first on the DVE FIFO:
                # nv = (p_n + bhh_n) * r
                nc.vector.scalar_tensor_tensor(
                    t_nv[:], p_n[:], b_hn, t_rz[:, 0:B], op0=ALU.add, op1=ALU.mult
                )
                # ni = nv + (xg_n[t] + bih_n)
                nc.vector.tensor_add(t_ni[:], t_nv[:], xgn[:, t * B : (t + 1) * B])
                # zh = z * h   (off chain)
                nc.vector.tensor_mul(t_zh[:], t_rz[:, B : 2 * B], h_mat)
                nc.scalar.activation(t_n[:], t_ni[:], AF.Tanh)
                # z2n = (1-z)*n  — last chain op; next step's matmuls take
                # (zh, z2n) directly, h' materializes off-chain for storage
                nc.vector.tensor_mul(t_z2n[:], t_z2[:], t_n[:])
                nc.vector.tensor_add(h_out, t_z2n[:], t_zh[:])
                return t_zh, t_z2n

            # encoder
            parts, h_mat = [s_h0[:]], s_h0[:]
            for t in range(L):
                h_out = s_encT[:, t * B : (t + 1) * B]
                zh, z2n = gru_step(parts, h_mat, s_eWrz, s_eWhhT, s_xTa, s_xgn,
                                   t, bcol(0), h_out)
                parts, h_mat = [zh[:], z2n[:]], h_out

            # select last state: lastT[h,b] = sum_t encT[h,(t,b)] * mask[t,b]
            s_sel = state.tile([128, L * B], f32, tag="s_sel")
            nc.vector.tensor_mul(s_sel[:], s_encT[:], s_mask[:])
            w = L * B
            while w > B:
                w //= 2
                nc.vector.tensor_add(s_sel[:, 0:w], s_sel[:, 0:w], s_sel[:, w : 2 * w])
            # dec_h0 = ds_W @ last + ds_b
            s_dh0 = state.tile([H, B], f32, tag="s_dh0")
            p_h0 = psn.tile([H, B], f32, tag="p_n")
            nc.tensor.matmul(p_h0[:], s_dsWT[:], s_sel[:, 0:B], start=True, stop=True)
            nc.scalar.activation(s_dh0[:], p_h0[:], AF.Identity, bias=bcol(4))

            # decoder: states chunked, one tile per CH steps, column = b*q + tl
            dec_chunks = []
            dec_views = []
            for c in range(NCHUNK):
                q = min(CH, TD - c * CH)
                tl_c = state.tile([H, B * q], f32, tag=f"s_dec{c}")
                dec_chunks.append((tl_c, q))
                dec_views.append(tl_c.rearrange("p (b q) -> p b q", q=q))

            chunk_lhsr = {}

            def emit_pair(c, vb):
                tl_c, q = dec_chunks[c]
                rows = B * q
                if c not in chunk_lhsr:
                    # one rounded (f32r) copy of the chunk's states for the PE
                    lhsr = gates.tile([H, rows], f32r, tag="lhsr")
                    nc.scalar.activation(lhsr[:], tl_c[:], AF.Identity)
                    chunk_lhsr[c] = lhsr
                lhs = chunk_lhsr[c][:]
                v0 = vb * VW
                p = pbig.tile([rows, VW], f32, tag="pbig")
                o_t = stage.tile([rows, VW], f32, tag="o_t")
                nc.tensor.matmul(
                    p[:], lhs, s_logWT[:, v0 : v0 + VW], start=True, stop=True
                )
                # psum -> sbuf move doubles as the bias add
                nc.vector.tensor_add(o_t[:], p[:], s_logb[0:rows, v0 : v0 + VW])
                nc.sync.dma_start(
                    out_v[:, c * CH : c * CH + q, v0 : v0 + VW], o_t[:]
                )

            # logits pairs trickle out at ~2 per decoder step, one chunk
            # behind the scan, so they fill engine-FIFO gaps instead of
            # stalling the recurrence
            queue = []
            parts, h_mat = [s_dh0[:]], s_dh0[:]
            for t in range(TD):
                c, tl = t // CH, t % CH
                h_out = dec_views[c][:, :, tl]
                zh, z2n = gru_step(parts, h_mat, s_dWrz, s_dWhhT, s_yTa, s_ygn,
                                   t, bcol(2), h_out)
                parts, h_mat = [zh[:], z2n[:]], h_out
                if tl == dec_chunks[c][1] - 1 and c > 0:
                    queue.extend((c - 1, vb) for vb in range(NVB))
                for k in range(3):
                    if queue:
                        emit_pair(*queue.pop(0))
            queue.extend((NCHUNK - 1, vb) for vb in range(NVB))
            for k, (c, vb) in enumerate(queue):
                emit_pair(c, vb)

    nc.finalize()
    return nc


def _prep_inputs(inp, out, emb_inp, emb_out, enc_Wih, enc_Whh, enc_bih, enc_bhh,
                 ds_W, ds_b, dec_Wih, dec_Whh, dec_bih, dec_bhh, log_W, log_b):
    f = np.float32
    inp = np.asarray(inp)
    out = np.asarray(out)
    emb_inp = np.asarray(emb_inp, f)
    emb_out = np.asarray(emb_out, f)

    def aug_acts(tok_emb):                  # [B, T, E] -> [E+1, T*B]
        b, t, e = tok_emb.shape
        a = np.empty((e + 1, t * b), f)
        a[:e] = tok_emb.transpose(2, 1, 0).reshape(e, t * b)
        a[e] = 1.0
        return a

    xTa = aug_acts(emb_inp[inp])            # [65, 2048]
    yTa = aug_acts(emb_out[out[:, :-1]])    # [65, 2016]

    lengths = np.minimum((inp != EOS_IX).sum(axis=1), L - 1).astype(np.int64)
    mask = np.zeros((L, B), f)
    mask[lengths, np.arange(B)] = 1.0

    enc_Wih = np.asarray(enc_Wih, f)
    dec_Wih = np.asarray(dec_Wih, f)
    enc_bih = np.asarray(enc_bih, f)
    enc_bhh = np.asarray(enc_bhh, f)
    dec_bih = np.asarray(dec_bih, f)
    dec_bhh = np.asarray(dec_bhh, f)

    def aug_wrz(Wih, bih, bhh):             # -> [E+1, 2H] with bias row
        a = np.empty((E + 1, 2 * H), f)
        a[:E] = Wih[0 : 2 * H].T
        a[E] = bih[0 : 2 * H] + bhh[0 : 2 * H]
        return a

    biases = np.zeros((H, 5), f)
    biases[:, 0] = enc_bhh[2 * H :]
    biases[:, 1] = enc_bih[2 * H :]
    biases[:, 2] = dec_bhh[2 * H :]
    biases[:, 3] = dec_bih[2 * H :]
    biases[:, 4] = np.asarray(ds_b, f)

    common = {
        "xTa": xTa,
        "yTa": yTa,
        "mask1": np.ascontiguousarray(mask.reshape(1, L * B)),
        "eWrz": aug_wrz(enc_Wih, enc_bih, enc_bhh),
        "dWrz": aug_wrz(dec_Wih, dec_bih, dec_bhh),
        "eWn": np.ascontiguousarray(enc_Wih[2 * H :].T),
        "dWn": np.ascontiguousarray(dec_Wih[2 * H :].T),
        "eWhhT": np.ascontiguousarray(np.asarray(enc_Whh, f).T),
        "dWhhT": np.ascontiguousarray(np.asarray(dec_Whh, f).T),
        "dsWT": np.ascontiguousarray(np.asarray(ds_W, f).T),
        "biases": biases,
    }
    log_W = np.asarray(log_W, f)
    log_b = np.asarray(log_b, f)
    in_maps = []
    for c in range(NCORES):
        m = dict(common)
        m["logWT"] = np.ascontiguousarray(log_W[c * VS : (c + 1) * VS, :].T)
        m["logb1"] = np.ascontiguousarray(log_b[c * VS : (c + 1) * VS]).reshape(1, VS)
        in_maps.append(m)
    return in_maps


def _run(in_maps, **spmd_kwargs):
    from concourse.bass_utils import run_bass_kernel_spmd

    if "nc" not in _CACHE:
        _CACHE["nc"] = _build()
    return run_bass_kernel_spmd(
        _CACHE["nc"], in_maps, core_ids=list(range(NCORES)), **spmd_kwargs
    )


def _assemble(results):
    res = np.empty((B, L, V), np.float32)
    onehot = np.zeros(V, np.float32)
    onehot[BOS_IX] = 1.0
    res[:, 0, :] = np.log(onehot + np.float32(1e-9), dtype=np.float32)[None, :]
    for c in range(NCORES):
        res[:, 1:, c * VS : (c + 1) * VS] = results[c]["o"].reshape(B, TD, VS)
    return res


def kernel(**inputs):
    in_maps = _prep_inputs(**inputs)
    r = _run(in_maps)
    return _assemble(r.results)


def kernel_profiled(trace_cores=None, **inputs):
    """Like kernel() but returns (output, BassKernelResults) with a trace."""
    in_maps = _prep_inputs(**inputs)
    r = _run(in_maps, trace=True, trace_cores=trace_cores or [0])
    return _assemble(r.results), r


# revision 46
# speedup vs baseline: 1.0078x; 1.0078x over previous
"""Trainium2 Bass kernel for the seq2seq GRU (encoder/decoder + vocab logits).

Strategy (8 NeuronCores):
  - The tiny GRU scans (B=32, H=128) are replicated on every core.
  - The dominant cost — the vocab projection [2016,128] @ [128,32000] and its
    ~262 MB f32 output write — is sharded over the vocab dim: each core owns
    V/8 = 4000 columns and writes its slab [32, 63, 4000] of the final output
    directly.
  - Decoder hidden states are stored in one tile per 4 timesteps so the vocab
    projection (float32r matmuls) and the output DMA stream overlap with the
    rest of the decoder scan.
  - r/z gate biases are folded into the input-side matmuls via an augmented
    ones-row; (1-z) comes from a second sigmoid with scale=-1.
  - Host only does embedding gathers, weight transposes, sharding and the
    constant first-column (BOS one-hot log) of the output.
"""

import numpy as np

EOS_IX = 2
BOS_IX = 1
V = 32000
E = 64
H = 128
B = 32
L = 64            # encoder length and output length
TD = L - 1        # 63 decoder steps
M = B * TD        # 2016 decoder tokens
NCORES = 8
VS = V // NCORES  # 4000
VW = 500          # vocab tile width (4000 = 8 * 500)
NVB = VS // VW    # 8
CH = 4            # decoder steps per chunk
NCHUNK = (TD + CH - 1) // CH          # 16 (last chunk has 3 steps)

_CACHE = {}


def _build():
    import concourse.bass as bass
    import concourse.mybir as mybir
    import concourse.tile as tile
    from concourse import bacc

    f32 = mybir.dt.float32
    f32r = mybir.dt.float32r
    AF = mybir.ActivationFunctionType
    ALU = mybir.AluOpType

    nc = bacc.Bacc(None, target_bir_lowering=False)

    xTa = nc.dram_tensor("xTa", [E + 1, L * B], f32, kind="ExternalInput")
    yTa = nc.dram_tensor("yTa", [E + 1, TD * B], f32, kind="ExternalInput")
    mask1 = nc.dram_tensor("mask1", [1, L * B], f32, kind="ExternalInput")
    eWrz = nc.dram_tensor("eWrz", [E + 1, 2 * H], f32, kind="ExternalInput")
    dWrz = nc.dram_tensor("dWrz", [E + 1, 2 * H], f32, kind="ExternalInput")
    eWn = nc.dram_tensor("eWn", [E, H], f32, kind="ExternalInput")
    dWn = nc.dram_tensor("dWn", [E, H], f32, kind="ExternalInput")
    eWhhT = nc.dram_tensor("eWhhT", [H, 3 * H], f32, kind="ExternalInput")
    dWhhT = nc.dram_tensor("dWhhT", [H, 3 * H], f32, kind="ExternalInput")
    dsWT = nc.dram_tensor("dsWT", [H, H], f32, kind="ExternalInput")
    biases = nc.dram_tensor("biases", [H, 5], f32, kind="ExternalInput")
    logWT = nc.dram_tensor("logWT", [H, VS], f32r, kind="ExternalInput")
    logb1 = nc.dram_tensor("logb1", [1, VS], f32, kind="ExternalInput")
    out = nc.dram_tensor("o", [M, VS], f32, kind="ExternalOutput")
    out_v = out.rearrange("(b t) v -> b t v", t=TD)   # [B, TD, VS]

    def bcast_ap(dram_t, n):
        # read the single row of dram_t into all 128 partitions (stride-0)
        ap = dram_t[:]
        return bass.AP(tensor=ap.tensor, offset=ap.offset, ap=[[0, 128], [1, n]])

    with tile.TileContext(nc) as tc:
        with (
            tc.tile_pool(name="state", bufs=1) as state,
            tc.tile_pool(name="gates", bufs=8) as gates,
            tc.tile_pool(name="stage", bufs=40) as stage,
            tc.tile_pool(name="psr", bufs=3, space="PSUM") as psr,
            tc.tile_pool(name="psn", bufs=3, space="PSUM") as psn,
            tc.tile_pool(name="pbig", bufs=2, space="PSUM") as pbig,
        ):
            # ---- load everything ----
            s_xTa = state.tile([E + 1, L * B], f32, tag="s_xTa")
            s_yTa = state.tile([E + 1, TD * B], f32, tag="s_yTa")
            s_mask = state.tile([128, L * B], f32, tag="s_mask")
            s_eWrz = state.tile([E + 1, 2 * H], f32, tag="s_eWrz")
            s_dWrz = state.tile([E + 1, 2 * H], f32, tag="s_dWrz")
            s_eWn = state.tile([E, H], f32, tag="s_eWn")
            s_dWn = state.tile([E, H], f32, tag="s_dWn")
            s_eWhhT = state.tile([H, 3 * H], f32, tag="s_eWhhT")
            s_dWhhT = state.tile([H, 3 * H], f32, tag="s_dWhhT")
            s_dsWT = state.tile([H, H], f32, tag="s_dsWT")
            s_bias = state.tile([H, 5], f32, tag="s_bias")
            s_logWT = state.tile([H, VS], f32r, tag="s_logWT")
            s_logb = state.tile([128, VS], f32, tag="s_logb")
            nc.sync.dma_start(s_xTa[:], xTa[:])
            nc.sync.dma_start(s_yTa[:], yTa[:])
            nc.sync.dma_start(s_mask[:], bcast_ap(mask1, L * B))
            nc.sync.dma_start(s_eWrz[:], eWrz[:])
            nc.sync.dma_start(s_dWrz[:], dWrz[:])
            nc.sync.dma_start(s_eWn[:], eWn[:])
            nc.sync.dma_start(s_dWn[:], dWn[:])
            nc.sync.dma_start(s_eWhhT[:], eWhhT[:])
            nc.sync.dma_start(s_dWhhT[:], dWhhT[:])
            nc.sync.dma_start(s_dsWT[:], dsWT[:])
            nc.sync.dma_start(s_bias[:], biases[:])
            nc.sync.dma_start(s_logWT[:], logWT[:])
            nc.sync.dma_start(s_logb[:], bcast_ap(logb1, VS))

            # bias cols: 0 enc bhh_n, 1 enc bih_n, 2 dec bhh_n, 3 dec bih_n, 4 ds_b
            def bcol(k):
                return s_bias[:, k : k + 1]

            # ---- precompute n-gate input-side activations xg_n = Wih_n @ x + bih_n
            s_xgn = state.tile([H, L * B], f32, tag="s_xgn")
            s_ygn = state.tile([H, TD * B], f32, tag="s_ygn")
            for nb in range(4):
                p = pbig.tile([H, 512], f32, tag="pbig")
                nc.tensor.matmul(
                    p[:], s_eWn[:], s_xTa[:E, nb * 512 : (nb + 1) * 512],
                    start=True, stop=True,
                )
                nc.scalar.activation(
                    s_xgn[:, nb * 512 : (nb + 1) * 512], p[:], AF.Identity,
                    bias=bcol(1),
                )
            for nb in range(4):
                p = pbig.tile([H, 504], f32, tag="pbig")
                nc.tensor.matmul(
                    p[:], s_dWn[:], s_yTa[:E, nb * 504 : (nb + 1) * 504],
                    start=True, stop=True,
                )
                nc.scalar.activation(
                    s_ygn[:, nb * 504 : (nb + 1) * 504], p[:], AF.Identity,
                    bias=bcol(3),
                )

            # ---- GRU scans ----
            s_encT = state.tile([H, L * B], f32, tag="s_encT")
            s_h0 = state.tile([H, B], f32, tag="s_h0")
            nc.vector.memset(s_h0[:], 0.0)

            def gru_step(h_parts, h_mat, wrz, whhT, xa, xgn, t, b_hn, h_out):
                """h_out <- GRU(h, x_t). The previous state enters the matmuls
                as its two partial products (z*h, (1-z)*n) so the final h'
                materialization stays off the recurrence-critical path.
                h_parts: rhs list for the Whh matmuls; h_mat: materialized h."""
                x_t = xa[:, t * B : (t + 1) * B]
                p_rz = psr.tile([H, 2 * B], f32, tag="p_rz")
                p_n = psn.tile([H, B], f32, tag="p_n")
                # input-side parts + folded biases (off critical path)
                nc.tensor.matmul(p_rz[:, 0:B], wrz[:, 0:H], x_t, start=True, stop=False)
                nc.tensor.matmul(p_rz[:, B : 2 * B], wrz[:, H : 2 * H], x_t, start=True, stop=False)
                # recurrent parts accumulate on top (one per h-part)
                for i, hp in enumerate(h_parts):
                    last = i == len(h_parts) - 1
                    nc.tensor.matmul(p_rz[:, 0:B], whhT[:, 0:H], hp, start=False, stop=last)
                    nc.tensor.matmul(p_rz[:, B : 2 * B], whhT[:, H : 2 * H], hp, start=False, stop=last)
                    nc.tensor.matmul(p_n[:], whhT[:, 2 * H : 3 * H], hp, start=(i == 0), stop=last)
                t_rz = gates.tile([H, 2 * B], f32, tag="t_rz")
                t_z2 = gates.tile([H, B], f32, tag="t_z2")
                t_zh = gates.tile([H, B], f32, tag="t_zh")
                t_nv = gates.tile([H, B], f32, tag="t_nv")
                t_ni = gates.tile([H, B], f32, tag="t_ni")
                t_n = gates.tile([H, B], f32, tag="t_n")
                t_z2n = gates.tile([H, B], f32, tag="t_z2n")
                # r|z = sigmoid(p_rz);  z2 = 1-z = sigmoid(-p_z)
                nc.scalar.activation(t_rz[:], p_rz[:], AF.Sigmoid)
                nc.scalar.activation(t_z2[:], p_rz[:, B : 2 * B], AF.Sigmoid, scale=-1.0)
                # chain ops first on the DVE FIFO:
                # nv = (p_n + bhh_n) * r
                nc.vector.scalar_tensor_tensor(
                    t_nv[:], p_n[:], b_hn, t_rz[:, 0:B], op0=ALU.add, op1=ALU.mult
                )
                # ni = nv + (xg_n[t] + bih_n)
                nc.vector.tensor_add(t_ni[:], t_nv[:], xgn[:, t * B : (t + 1) * B])
                # zh = z * h   (off chain)
                nc.vector.tensor_mul(t_zh[:], t_rz[:, B : 2 * B], h_mat)
                nc.scalar.activation(t_n[:], t_ni[:], AF.Tanh)
                # z2n = (1-z)*n  — last chain op; next step's matmuls take
                # (zh, z2n) directly, h' materializes off-chain for storage
                nc.vector.tensor_mul(t_z2n[:], t_z2[:], t_n[:])
                nc.vector.tensor_add(h_out, t_z2n[:], t_zh[:])
                return t_zh, t_z2n

            # encoder
            parts, h_mat = [s_h0[:]], s_h0[:]
            for t in range(L):
                h_out = s_encT[:, t * B : (t + 1) * B]
                zh, z2n = gru_step(parts, h_mat, s_eWrz, s_eWhhT, s_xTa, s_xgn,
                                   t, bcol(0), h_out)
                parts, h_mat = [zh[:], z2n[:]], h_out

            # select last state: lastT[h,b] = sum_t encT[h,(t,b)] * mask[t,b]
            s_sel = state.tile([128, L * B], f32, tag="s_sel")
            nc.vector.tensor_mul(s_sel[:], s_encT[:], s_mask[:])
            w = L * B
            while w > B:
                w //= 2
                nc.vector.tensor_add(s_sel[:, 0:w], s_sel[:, 0:w], s_sel[:, w : 2 * w])
            # dec_h0 = ds_W @ last + ds_b
            s_dh0 = state.tile([H, B], f32, tag="s_dh0")
            p_h0 = psn.tile([H, B], f32, tag="p_n")
            nc.tensor.matmul(p_h0[:], s_dsWT[:], s_sel[:, 0:B], start=True, stop=True)
            nc.scalar.activation(s_dh0[:], p_h0[:], AF.Identity, bias=bcol(4))

            # decoder: states chunked, one tile per CH steps, column = b*q + tl
            dec_chunks = []
            dec_views = []
            for c in range(NCHUNK):
                q = min(CH, TD - c * CH)
                tl_c = state.tile([H, B * q], f32, tag=f"s_dec{c}")
                dec_chunks.append((tl_c, q))
                dec_views.append(tl_c.rearrange("p (b q) -> p b q", q=q))

            chunk_lhsr = {}

            def emit_pair(c, vb):
                tl_c, q = dec_chunks[c]
                rows = B * q
                if c not in chunk_lhsr:
                    # one rounded (f32r) copy of the chunk's states for the PE
                    lhsr = gates.tile([H, rows], f32r, tag="lhsr")
                    nc.scalar.activation(lhsr[:], tl_c[:], AF.Identity)
                    chunk_lhsr[c] = lhsr
                lhs = chunk_lhsr[c][:]
                v0 = vb * VW
                p = pbig.tile([rows, VW], f32, tag="pbig")
                o_t = stage.tile([rows, VW], f32, tag="o_t")
                nc.tensor.matmul(
                    p[:], lhs, s_logWT[:, v0 : v0 + VW], start=True, stop=True
                )
                # psum -> sbuf move doubles as the bias add
                nc.vector.tensor_add(o_t[:], p[:], s_logb[0:rows, v0 : v0 + VW])
                nc.sync.dma_start(
                    out_v[:, c * CH : c * CH + q, v0 : v0 + VW], o_t[:]
                )

            # logits pairs trickle out at ~2 per decoder step, one chunk
            # behind the scan, so they fill engine-FIFO gaps instead of
            # stalling the recurrence
            queue = []
            parts, h_mat = [s_dh0[:]], s_dh0[:]
            for t in range(TD):
                c, tl = t // CH, t % CH
                h_out = dec_views[c][:, :, tl]
                zh, z2n = gru_step(parts, h_mat, s_dWrz, s_dWhhT, s_yTa, s_ygn,
                                   t, bcol(2), h_out)
                parts, h_mat = [zh[:], z2n[:]], h_out
                if tl == dec_chunks[c][1] - 1 and c > 0:
                    queue.extend((c - 1, vb) for vb in range(NVB))
                for k in range(3):
                    if queue:
                        emit_pair(*queue.pop(0))
            queue.extend((NCHUNK - 1, vb) for vb in range(NVB))
            for k, (c, vb) in enumerate(queue):
                emit_pair(c, vb)

    nc.finalize()
    return nc


def _prep_inputs(inp, out, emb_inp, emb_out, enc_Wih, enc_Whh, enc_bih, enc_bhh,
                 ds_W, ds_b, dec_Wih, dec_Whh, dec_bih, dec_bhh, log_W, log_b):
    f = np.float32
    inp = np.asarray(inp)
    out = np.asarray(out)
    emb_inp = np.asarray(emb_inp, f)
    emb_out = np.asarray(emb_out, f)

    def aug_acts(tok_emb):                  # [B, T, E] -> [E+1, T*B]
        b, t, e = tok_emb.shape
        a = np.empty((e + 1, t * b), f)
        a[:e] = tok_emb.transpose(2, 1, 0).reshape(e, t * b)
        a[e] = 1.0
        return a

    xTa = aug_acts(emb_inp[inp])            # [65, 2048]
    yTa = aug_acts(emb_out[out[:, :-1]])    # [65, 2016]

    lengths = np.minimum((inp != EOS_IX).sum(axis=1), L - 1).astype(np.int64)
    mask = np.zeros((L, B), f)
    mask[lengths, np.arange(B)] = 1.0

    enc_Wih = np.asarray(enc_Wih, f)
    dec_Wih = np.asarray(dec_Wih, f)
    enc_bih = np.asarray(enc_bih, f)
    enc_bhh = np.asarray(enc_bhh, f)
    dec_bih = np.asarray(dec_bih, f)
    dec_bhh = np.asarray(dec_bhh, f)

    def aug_wrz(Wih, bih, bhh):             # -> [E+1, 2H] with bias row
        a = np.empty((E + 1, 2 * H), f)
        a[:E] = Wih[0 : 2 * H].T
        a[E] = bih[0 : 2 * H] + bhh[0 : 2 * H]
        return a

    biases = np.zeros((H, 5), f)
    biases[:, 0] = enc_bhh[2 * H :]
    biases[:, 1] = enc_bih[2 * H :]
    biases[:, 2] = dec_bhh[2 * H :]
    biases[:, 3] = dec_bih[2 * H :]
    biases[:, 4] = np.asarray(ds_b, f)

    common = {
        "xTa": xTa,
        "yTa": yTa,
        "mask1": np.ascontiguousarray(mask.reshape(1, L * B)),
        "eWrz": aug_wrz(enc_Wih, enc_bih, enc_bhh),
        "dWrz": aug_wrz(dec_Wih, dec_bih, dec_bhh),
        "eWn": np.ascontiguousarray(enc_Wih[2 * H :].T),
        "dWn": np.ascontiguousarray(dec_Wih[2 * H :].T),
        "eWhhT": np.ascontiguousarray(np.asarray(enc_Whh, f).T),
        "dWhhT": np.ascontiguousarray(np.asarray(dec_Whh, f).T),
        "dsWT": np.ascontiguousarray(np.asarray(ds_W, f).T),
        "biases": biases,
    }
    log_W = np.asarray(log_W, f)
    log_b = np.asarray(log_b, f)
    in_maps = []
    for c in range(NCORES):
        m = dict(common)
        m["logWT"] = np.ascontiguousarray(log_W[c * VS : (c + 1) * VS, :].T)
        m["logb1"] = np.ascontiguousarray(log_b[c * VS : (c + 1) * VS]).reshape(1, VS)
        in_maps.append(m)
    return in_maps


def _run(in_maps, **spmd_kwargs):
    from concourse.bass_utils import run_bass_kernel_spmd

    if "nc" not in _CACHE:
        _CACHE["nc"] = _build()
    return run_bass_kernel_spmd(
        _CACHE["nc"], in_maps, core_ids=list(range(NCORES)), **spmd_kwargs
    )


def _assemble(results):
    res = np.empty((B, L, V), np.float32)
    onehot = np.zeros(V, np.float32)
    onehot[BOS_IX] = 1.0
    res[:, 0, :] = np.log(onehot + np.float32(1e-9), dtype=np.float32)[None, :]
    for c in range(NCORES):
        res[:, 1:, c * VS : (c + 1) * VS] = results[c]["o"].reshape(B, TD, VS)
    return res


def kernel(**inputs):
    in_maps = _prep_inputs(**inputs)
    r = _run(in_maps)
    return _assemble(r.results)


def kernel_profiled(trace_cores=None, **inputs):
    """Like kernel() but returns (output, BassKernelResults) with a trace."""
    in_maps = _prep_inputs(**inputs)
    r = _run(in_maps, trace=True, trace_cores=trace_cores or [0])
    return _assemble(r.results), r
